# revision 8
# baseline (speedup 1.0000x reference)
"""JKNet (5-layer GCN + JumpingKnowledge-max + linear head) on 8 Trainium2 cores.

Strategy (dst-sharded message passing, v3):
  - Nodes are sharded contiguously across 8 cores (12500 per core).
  - Edges (plus explicit self-loops carrying the 1/deg self term) are
    partitioned by destination shard, grouped by 128-row destination tile,
    bucketed by source range (4 buckets of <=25000 rows so indices fit in
    int16 for the HW dma_gather), and sorted by source inside each
    (tile, bucket) segment for locality.  Segments are padded to multiples
    of 128 with zero-norm repeats.
  - A prologue casts each core's x shard to bf16 (SWDGE dma cast) and
    AllGathers the full bf16 replica; per-layer AllGathers rebuild it.
  - Aggregation for a group of G_TILES destination tiles: one dma_gather per
    (group, bucket) pulls all the group's source rows for that bucket into
    SBUF ([128, ncb, 128] chunk-major), then per 128-edge chunk a one-hot
    selection matrix S[e, dst] = norm_e * (iota == dst_e) is built with one
    fused tensor_scalar op (alternating DVE / GpSimd to balance engines) and
    q^T[feat, dst] += msgs^T @ S accumulates on the PE (bf16, fp32 PSUM).
  - Per tile: q^T -> SBUF (ACT copy, bf16), W matmul (bf16), fused BN+ReLU
    on ACT (bf16 out), JumpingKnowledge running max (DVE, bf16), PE
    transpose back to node-major, DMA to the AllGather input buffer.
  - Head: logits = hmax^T.T @ lin_w per tile (bf16), + bias, log_softmax
    (fp32), DMA the core's [12500, 40] shard out.

The per-(tile,bucket) chunk counts are data dependent; the Bass program is
generated per problem instance (shared by all 8 cores -- chunk counts are
maxed over cores and shorter cores are padded with zero-norm edges).
"""

import math
import os

import numpy as np

import concourse.bass as bass
import concourse.mybir as mybir
import concourse.tile as tile
from concourse import bacc
from concourse.bass_utils import run_bass_kernel_spmd
from concourse.masks import make_identity

P = 128          # partitions / feature dim / edge-chunk size
NCORES = 8
BN_EPS = 1e-5
G_TILES = 8      # dst tiles per gather group
NB = 4           # source-range buckets (int16 index limit)
S_POOL_MOD = 3   # every S_POOL_MOD-th S-build goes to GpSimd (0 = all DVE)


# ---------------------------------------------------------------- host prep
def preprocess_edges(edge_index, n_nodes, ncores=NCORES):
    """Partition edges (incl. self-loops) by destination shard, bucket by
    source range, group by 128-row destination tile.

    Returns (per_core, meta) where per_core has:
      'edst' [P, k_total] f32, 'enrm' [P, k_total] f32   (chunk-major)
      'egthr' [P, 8*k_total] int16  (dma_gather wrapped+replicated indices,
                                     concatenated per (group, bucket) call)
    and meta describes the static chunk structure.
    """
    row = np.asarray(edge_index[0], dtype=np.int64)   # dst
    col = np.asarray(edge_index[1], dtype=np.int64)   # src
    deg = np.bincount(row, minlength=n_nodes).astype(np.float64) + 1.0
    dinv = (1.0 / np.sqrt(deg)).astype(np.float32)

    dst_all = np.concatenate([row, np.arange(n_nodes, dtype=np.int64)])
    src_all = np.concatenate([col, np.arange(n_nodes, dtype=np.int64)])
    nrm_all = np.concatenate([dinv[row] * dinv[col], dinv * dinv]).astype(np.float32)

    sh = n_nodes // ncores
    t_tiles = math.ceil(sh / P)
    bsz = math.ceil(n_nodes / NB)          # bucket size (rows per sub-table)
    core_of = dst_all // sh

    # per core: edges sorted by (tile, bucket, src)
    cores = []
    counts = np.zeros((ncores, t_tiles, NB), dtype=np.int64)
    for c in range(ncores):
        m = core_of == c
        d = (dst_all[m] - c * sh).astype(np.int64)
        s = src_all[m]
        w = nrm_all[m]
        tid = d // P
        din = (d % P).astype(np.float32)
        bid = s // bsz
        order = np.lexsort((s, bid, tid))
        tid, bid, din, s, w = tid[order], bid[order], din[order], s[order], w[order]
        for t in range(t_tiles):
            msk = tid == t
            counts[c, t] = np.bincount(bid[msk], minlength=NB)
        cores.append((tid, bid, din, s, w))

    # shared chunk counts per (tile, bucket)
    kt_b = np.zeros((t_tiles, NB), dtype=np.int64)
    for t in range(t_tiles):
        for b in range(NB):
            kt_b[t, b] = int(math.ceil(counts[:, t, b].max() / P))
    groups = [list(range(g * G_TILES, min((g + 1) * G_TILES, t_tiles)))
              for g in range(math.ceil(t_tiles / G_TILES))]

    # global chunk order: per (group, bucket) call, tiles in group order.
    calls = []          # (grp_idx, b, [tiles], ncb, chunk0, col0)
    chunk0_of = {}
    cnum = 0
    col0 = 0
    for gi, grp in enumerate(groups):
        for b in range(NB):
            ncb = int(sum(kt_b[t, b] for t in grp))
            if ncb == 0:
                continue
            for t in grp:
                chunk0_of[(t, b)] = cnum + int(
                    sum(kt_b[tt, b] for tt in grp if tt < t))
            calls.append((gi, b, list(grp), ncb, cnum, col0))
            cnum += ncb
            col0 += ncb * 8          # 128 idx per chunk / 16 wrap = 8 cols
    k_total = cnum

    per_core = []
    for c in range(ncores):
        tid, bid, din, s, w = cores[c]
        dst_f = np.zeros(k_total * P, dtype=np.float32)
        nrm_f = np.zeros(k_total * P, dtype=np.float32)
        idx_f = np.zeros(k_total * P, dtype=np.int64)
        # segment starts per (t, b) in the sorted arrays
        seg_start = {}
        pos = 0
        for t in range(t_tiles):
            for b in range(NB):
                seg_start[(t, b)] = pos
                pos += int(counts[c, t, b])
        for (t, b), c0 in chunk0_of.items():
            n = int(counts[c, t, b])
            a = seg_start[(t, b)]
            o = c0 * P
            cap = int(kt_b[t, b]) * P
            idx_f[o:o + n] = s[a:a + n] - b * bsz
            dst_f[o:o + n] = din[a:a + n]
            nrm_f[o:o + n] = w[a:a + n]
            if cap > n:
                idx_f[o + n:o + cap] = idx_f[o + n - 1] if n else 0
        # dma_gather wrapped layout per call, replicated across partitions
        egthr = np.zeros((P, col0), dtype=np.int16)
        for (_gi, _b, _grp, ncb, cn0, cc0) in calls:
            seg = idx_f[cn0 * P:(cn0 + ncb) * P]
            wcols = seg.reshape(-1, 16).T.astype(np.int16)   # [16, ncb*8]
            egthr[:, cc0:cc0 + ncb * 8] = np.tile(wcols, (P // 16, 1))
        per_core.append({
            "edst": np.ascontiguousarray(dst_f.reshape(k_total, P).T),
            "enrm": np.ascontiguousarray(nrm_f.reshape(k_total, P).T),
            "egthr": np.ascontiguousarray(egthr),
        })
    meta = dict(t_tiles=t_tiles, groups=groups, calls=calls,
                chunk0_of=chunk0_of, kt_b=kt_b, k_total=k_total,
                bsz=bsz, idx_cols=col0)
    return per_core, meta


# ---------------------------------------------------------------- program
def build_program(n_nodes, n_layers, n_cls, meta, ncores=NCORES):
    f32 = mybir.dt.float32
    bf16 = mybir.dt.bfloat16
    i16 = mybir.dt.int16
    sh = n_nodes // ncores
    t_tiles = meta["t_tiles"]
    groups = meta["groups"]
    calls = meta["calls"]
    chunk0_of = meta["chunk0_of"]
    kt_b = meta["kt_b"]
    k_total = meta["k_total"]
    bsz = meta["bsz"]
    idx_cols = meta["idx_cols"]
    max_ncb = max(c[3] for c in calls)

    nc = bacc.Bacc("TRN2", target_bir_lowering=False, debug=False,
                   num_devices=ncores)
    xb_t = nc.dram_tensor("xbf", [n_nodes, P], bf16, kind="ExternalInput")
    gth_t = nc.dram_tensor("egthr", [P, idx_cols], i16, kind="ExternalInput")
    dst_t = nc.dram_tensor("edst", [P, k_total], f32, kind="ExternalInput")
    nrm_t = nc.dram_tensor("enrm", [P, k_total], f32, kind="ExternalInput")
    w_t = nc.dram_tensor("conv_w", [n_layers, P, P], f32, kind="ExternalInput")
    cb_t = nc.dram_tensor("conv_b", [n_layers, P], f32, kind="ExternalInput")
    gam_t = nc.dram_tensor("bn_gamma", [n_layers, P], f32, kind="ExternalInput")
    bet_t = nc.dram_tensor("bn_beta", [n_layers, P], f32, kind="ExternalInput")
    mu_t = nc.dram_tensor("bn_mean", [n_layers, P], f32, kind="ExternalInput")
    var_t = nc.dram_tensor("bn_var", [n_layers, P], f32, kind="ExternalInput")
    lw_t = nc.dram_tensor("lin_w", [P, n_cls], f32, kind="ExternalInput")
    lb_t = nc.dram_tensor("lin_b_rep", [P, n_cls], f32, kind="ExternalInput")
    out_t = nc.dram_tensor("out", [sh, n_cls], f32, kind="ExternalOutput")

    hbuf = [nc.dram_tensor(f"hbuf{l}", [n_nodes, P], bf16, addr_space="Shared")
            for l in range(1, n_layers)]
    hbuf = [xb_t] + hbuf                     # layer-0 table = host-cast bf16 x
    ag_in = [nc.dram_tensor(f"ag_in{l}", [sh, P], bf16)
             for l in range(n_layers - 1)]
    rgroups = [list(range(ncores))]
    AF = mybir.ActivationFunctionType
    OP = mybir.AluOpType

    with tile.TileContext(nc) as tc:
        with tc.tile_pool(name="const", bufs=1) as cpool, \
             tc.tile_pool(name="edges", bufs=1) as epool, \
             tc.tile_pool(name="msgs", bufs=2) as mpool, \
             tc.tile_pool(name="spool", bufs=8) as spool, \
             tc.tile_pool(name="work", bufs=3) as wpool, \
             tc.tile_pool(name="psum", bufs=2, space="PSUM") as pspool, \
             tc.tile_pool(name="psumq", bufs=4, space="PSUM") as pqpool:

            # -------- resident edge data + constants
            gth_sb = epool.tile([P, idx_cols], i16)
            dst_sb = epool.tile([P, k_total], f32)
            nrm_sb = epool.tile([P, k_total], f32)
            nc.sync.dma_start(out=gth_sb[:], in_=gth_t[:])
            nc.sync.dma_start(out=dst_sb[:], in_=dst_t[:])
            nc.sync.dma_start(out=nrm_sb[:], in_=nrm_t[:])

            iota_i = cpool.tile([P, P], mybir.dt.int32)
            nc.gpsimd.iota(iota_i[:], pattern=[[1, P]], base=0, channel_multiplier=0)
            iota_b = cpool.tile([P, P], bf16)
            nc.vector.tensor_copy(iota_b[:], iota_i[:])
            ident = cpool.tile([P, P], bf16)
            make_identity(nc, ident[:])

            w_sb = []
            for l in range(n_layers):
                wf = cpool.tile([P, P], f32, tag=f"wf{l}")
                nc.sync.dma_start(out=wf[:], in_=w_t[l, :, :])
                wl = cpool.tile([P, P], bf16, tag=f"w{l}")
                nc.vector.tensor_copy(wl[:], wf[:])
                w_sb.append(wl)
            lwf = cpool.tile([P, n_cls], f32)
            nc.sync.dma_start(out=lwf[:], in_=lw_t[:])
            lw_sb = cpool.tile([P, n_cls], bf16)
            nc.vector.tensor_copy(lw_sb[:], lwf[:])
            lb_sb = cpool.tile([P, n_cls], f32)
            nc.sync.dma_start(out=lb_sb[:], in_=lb_t[:])

            # -------- BN constants per layer: scale s = gamma * rsqrt(var+eps)
            #          shift = s*(conv_b - mean) + beta      (feature-major [P,1])
            s_sb, sh_sb = [], []
            for l in range(n_layers):
                g_ = cpool.tile([P, 1], f32, tag=f"bng{l}")
                b_ = cpool.tile([P, 1], f32, tag=f"bnb{l}")
                m_ = cpool.tile([P, 1], f32, tag=f"bnm{l}")
                v_ = cpool.tile([P, 1], f32, tag=f"bnv{l}")
                cb_ = cpool.tile([P, 1], f32, tag=f"bnc{l}")
                nc.sync.dma_start(out=g_[:], in_=gam_t[l, :, None])
                nc.sync.dma_start(out=b_[:], in_=bet_t[l, :, None])
                nc.sync.dma_start(out=m_[:], in_=mu_t[l, :, None])
                nc.sync.dma_start(out=v_[:], in_=var_t[l, :, None])
                nc.sync.dma_start(out=cb_[:], in_=cb_t[l, :, None])
                ve = cpool.tile([P, 1], f32, tag=f"bnve{l}")
                nc.vector.tensor_scalar_add(ve[:], v_[:], BN_EPS)
                nc.scalar.sqrt(ve[:], ve[:])
                rv = cpool.tile([P, 1], f32, tag=f"bnrv{l}")
                nc.vector.reciprocal(rv[:], ve[:])
                s_ = cpool.tile([P, 1], f32, tag=f"bns{l}")
                nc.vector.tensor_tensor(out=s_[:], in0=g_[:], in1=rv[:], op=OP.mult)
                d_ = cpool.tile([P, 1], f32, tag=f"bnd{l}")
                nc.vector.tensor_tensor(out=d_[:], in0=cb_[:], in1=m_[:], op=OP.subtract)
                t_ = cpool.tile([P, 1], f32, tag=f"bnt{l}")
                nc.vector.tensor_tensor(out=t_[:], in0=d_[:], in1=s_[:], op=OP.mult)
                nc.vector.tensor_tensor(out=t_[:], in0=t_[:], in1=b_[:], op=OP.add)
                s_sb.append(s_)
                sh_sb.append(t_)

            hmax = epool.tile([P, t_tiles * P], bf16)
            nc.vector.memset(hmax[:], 0.0)

            # -------- layers
            scount = 0
            for l in range(n_layers):
                table = hbuf[l]
                for gi, grp in enumerate(groups):
                    gcalls = [c for c in calls if c[0] == gi]
                    mt = {}
                    for (_gi, b, _grp, ncb, cn0, cc0) in gcalls:
                        m = mpool.tile([P, max_ncb, P], bf16, tag=f"m{b}")
                        r0 = b * bsz
                        r1 = min((b + 1) * bsz, n_nodes)
                        nc.gpsimd.dma_gather(
                            m[:, :ncb, :], table[r0:r1, :],
                            gth_sb[:, cc0:cc0 + ncb * 8],
                            ncb * P, ncb * P, P, single_packet=False)
                        mt[b] = (m, cn0)
                    for t in grp:
                        ktt = int(kt_b[t].sum())
                        psq = pqpool.tile([P, P], f32, tag="q", space="PSUM")
                        j = 0
                        for b in range(NB):
                            kb = int(kt_b[t, b])
                            if kb == 0:
                                continue
                            m, cn0 = mt[b]
                            c0 = chunk0_of[(t, b)]
                            for jb in range(kb):
                                c = c0 + jb
                                s_tile = spool.tile([P, P], bf16, tag="sb")
                                eng = (nc.gpsimd if S_POOL_MOD and
                                       scount % S_POOL_MOD == 0 else nc.vector)
                                eng.tensor_scalar(
                                    out=s_tile[:], in0=iota_b[:],
                                    scalar1=dst_sb[:, c:c + 1],
                                    scalar2=nrm_sb[:, c:c + 1],
                                    op0=OP.is_equal, op1=OP.mult)
                                scount += 1
                                nc.tensor.matmul(
                                    psq[:], lhsT=m[:, c - cn0, :],
                                    rhs=s_tile[:],
                                    start=(j == 0), stop=(j == ktt - 1))
                                j += 1
                        q_sb = wpool.tile([P, P], bf16, tag="qT")
                        nc.scalar.copy(q_sb[:], psq[:])
                        ph = pspool.tile([P, P], f32, tag="h", space="PSUM")
                        nc.tensor.matmul(ph[:], lhsT=w_sb[l][:], rhs=q_sb[:],
                                         start=True, stop=True)
                        h_t = wpool.tile([P, P], bf16, tag="hT")
                        nc.scalar.activation(h_t[:], ph[:], AF.Relu,
                                             bias=sh_sb[l][:, :1],
                                             scale=s_sb[l][:, :1])
                        nc.vector.tensor_tensor(
                            out=hmax[:, t * P:(t + 1) * P],
                            in0=hmax[:, t * P:(t + 1) * P], in1=h_t[:], op=OP.max)
                        if l < n_layers - 1:
                            pt = pspool.tile([P, P], bf16, tag="t", space="PSUM")
                            nc.tensor.transpose(pt[:], h_t[:], ident[:])
                            hn = wpool.tile([P, P], bf16, tag="hn")
                            nc.scalar.copy(hn[:], pt[:])
                            rows = min(P, sh - t * P)
                            nc.sync.dma_start(out=ag_in[l][t * P:t * P + rows, :],
                                              in_=hn[:rows, :])
                if l < n_layers - 1:
                    nc.gpsimd.collective_compute(
                        "AllGather", OP.bypass, replica_groups=rgroups,
                        ins=[ag_in[l][:]], outs=[hbuf[l + 1][:]])

            # -------- head: logits + log_softmax
            for t in range(t_tiles):
                po = pspool.tile([P, n_cls], f32, tag="h", space="PSUM")
                nc.tensor.matmul(po[:], lhsT=hmax[:, t * P:(t + 1) * P],
                                 rhs=lw_sb[:], start=True, stop=True)
                z = wpool.tile([P, n_cls], f32, tag="z")
                nc.vector.tensor_tensor(out=z[:], in0=po[:], in1=lb_sb[:], op=OP.add)
                nm = wpool.tile([P, 1], f32, tag="nm")
                nc.vector.reduce_max(nm[:], z[:], axis=mybir.AxisListType.X,
                                     negate=True)
                ez = wpool.tile([P, n_cls], f32, tag="ez")
                nc.scalar.activation(ez[:], z[:], AF.Exp, bias=nm[:, :1], scale=1.0)
                ss = wpool.tile([P, 1], f32, tag="ss")
                nc.vector.reduce_sum(ss[:], ez[:], axis=mybir.AxisListType.X)
                ls = wpool.tile([P, 1], f32, tag="ls")
                nc.scalar.activation(ls[:], ss[:], AF.Ln)
                oz = wpool.tile([P, n_cls], f32, tag="oz")
                nc.vector.tensor_scalar(out=oz[:], in0=z[:],
                                        scalar1=nm[:, :1], scalar2=ls[:, :1],
                                        op0=OP.add, op1=OP.subtract)
                rows = min(P, sh - t * P)
                nc.sync.dma_start(out=out_t[t * P:t * P + rows, :],
                                  in_=oz[:rows, :])

    nc.compile()
    return nc


# ---------------------------------------------------------------- runner
def run(x, edge_index, conv_w, conv_b, bn_gamma, bn_beta, bn_mean, bn_var,
        lin_w, lin_b, *, trace=False):
    n_nodes, d = x.shape
    n_layers = conv_w.shape[0]
    n_cls = lin_w.shape[1]
    assert d == P and n_nodes % NCORES == 0
    sh = n_nodes // NCORES

    per_core, meta = preprocess_edges(edge_index, n_nodes)
    nc = build_program(n_nodes, n_layers, n_cls, meta)

    from ml_dtypes import bfloat16
    xbf = np.ascontiguousarray(
        np.asarray(x, dtype=np.float32).astype(bfloat16))
    shared = {
        "xbf": xbf,
        "conv_w": np.ascontiguousarray(np.asarray(conv_w, dtype=np.float32)),
        "conv_b": np.ascontiguousarray(np.asarray(conv_b, dtype=np.float32)),
        "bn_gamma": np.ascontiguousarray(np.asarray(bn_gamma, dtype=np.float32)),
        "bn_beta": np.ascontiguousarray(np.asarray(bn_beta, dtype=np.float32)),
        "bn_mean": np.ascontiguousarray(np.asarray(bn_mean, dtype=np.float32)),
        "bn_var": np.ascontiguousarray(np.asarray(bn_var, dtype=np.float32)),
        "lin_w": np.ascontiguousarray(np.asarray(lin_w, dtype=np.float32)),
        "lin_b_rep": np.ascontiguousarray(
            np.broadcast_to(np.asarray(lin_b, dtype=np.float32), (P, n_cls))),
    }
    in_maps = [dict(shared, **per_core[c]) for c in range(NCORES)]
    res = run_bass_kernel_spmd(nc, in_maps, list(range(NCORES)), trace=trace)
    out = np.concatenate([np.asarray(res.results[c]["out"])
                          for c in range(NCORES)], axis=0)
    return out, res


def kernel(x, edge_index, conv_w, conv_b, bn_gamma, bn_beta, bn_mean, bn_var,
           lin_w, lin_b):
    out, _ = run(x, edge_index, conv_w, conv_b, bn_gamma, bn_beta,
                 bn_mean, bn_var, lin_w, lin_b,
                 trace=bool(int(os.environ.get("JKNET_TRACE", "0"))))
    return out


# revision 10
# speedup vs baseline: 1.9867x; 1.9867x over previous
"""JKNet (5-layer GCN + JumpingKnowledge-max + linear head) on 8 Trainium2 cores.

Strategy (dst-sharded message passing, v4):
  - Nodes are sharded contiguously across 8 cores (12500 per core).
  - Self-loops are extracted: the self term for a destination tile reads the
    core's own shard rows with one contiguous bulk DMA (no per-row
    descriptors) and a diagonal S matrix.
  - Remaining edges are partitioned by destination shard, grouped by 128-row
    destination tile, and sorted by source inside each tile.  Per 128-edge
    chunk: one HW indirect DMA (one dynamic offset per output partition,
    ~1us of GpSimd descriptor generation - the critical resource), one fused
    DVE op building S[e, dst] = norm_e * (iota == dst_e) in bf16, and a bf16
    PE matmul accumulating q^T[feat, dst] += msgs^T @ S into fp32 PSUM.
  - All node-feature tables are bf16: x is cast host-side (xbf replica +
    per-core xself shard); per-layer AllGathers rebuild the bf16 replica.
  - Per tile: q^T -> SBUF (ACT copy, bf16), W matmul (bf16), fused BN+ReLU
    on ACT (bf16 out), JumpingKnowledge running max (DVE, bf16), PE
    transpose back to node-major, DMA to the AllGather input buffer.
  - Head: logits = hmax^T.T @ lin_w per tile (bf16), + bias, log_softmax
    (fp32), DMA the core's [12500, 40] shard out.

The per-destination-tile chunk counts are data dependent; the Bass program
is generated per problem instance (shared by all 8 cores -- per-tile chunk
counts are maxed over cores and shorter cores are padded with zero-norm
edges).
"""

import math
import os

import numpy as np

import concourse.bass as bass
import concourse.mybir as mybir
import concourse.tile as tile
from concourse import bacc
from concourse.bass_utils import run_bass_kernel_spmd
from concourse.masks import make_identity

P = 128          # partitions / feature dim / edge-chunk size
NCORES = 8
BN_EPS = 1e-5
G_TILES = 4      # dst tiles per issue group


# ---------------------------------------------------------------- host prep
def preprocess_edges(edge_index, n_nodes, ncores=NCORES):
    """Partition non-self edges by destination shard / 128-row dst tile.

    Returns (per_core, k_tiles):
      per_core: 'eidx' [P, K] int32, 'edst' [P, K] f32, 'enrm' [P, K] f32,
                'esnrm' [P, t_tiles] f32 (per-tile self-loop norms 1/deg)
      k_tiles:  per-dst-tile gather-chunk counts (shared across cores)
    """
    row = np.asarray(edge_index[0], dtype=np.int64)   # dst
    col = np.asarray(edge_index[1], dtype=np.int64)   # src
    deg = np.bincount(row, minlength=n_nodes).astype(np.float64) + 1.0
    dinv = (1.0 / np.sqrt(deg)).astype(np.float32)
    snrm_all = (dinv * dinv).astype(np.float32)       # self norm = 1/deg

    nrm_e = (dinv[row] * dinv[col]).astype(np.float32)

    sh = n_nodes // ncores
    t_tiles = math.ceil(sh / P)
    core_of = row // sh

    cores = []
    counts = np.zeros((ncores, t_tiles), dtype=np.int64)
    for c in range(ncores):
        m = core_of == c
        d = (row[m] - c * sh).astype(np.int64)
        s = col[m]
        w = nrm_e[m]
        tid = d // P
        din = (d % P).astype(np.float32)
        order = np.lexsort((s, tid))
        tid, din, s, w = tid[order], din[order], s[order], w[order]
        counts[c] = np.bincount(tid, minlength=t_tiles)
        cores.append((tid, din, s, w))

    k_tiles = [max(1, int(math.ceil(counts[:, t].max() / P))) for t in range(t_tiles)]
    k_total = sum(k_tiles)
    offs = np.concatenate([[0], np.cumsum(k_tiles)])

    per_core = []
    for c in range(ncores):
        tid, din, s, w = cores[c]
        idx_f = np.zeros(k_total * P, dtype=np.int32)
        dst_f = np.zeros(k_total * P, dtype=np.float32)
        nrm_f = np.zeros(k_total * P, dtype=np.float32)
        tstart = np.concatenate([[0], np.cumsum(counts[c])])
        for t in range(t_tiles):
            n = int(counts[c][t])
            a, b = int(tstart[t]), int(tstart[t] + n)
            o = int(offs[t]) * P
            idx_f[o:o + n] = s[a:b]
            dst_f[o:o + n] = din[a:b]
            nrm_f[o:o + n] = w[a:b]
            pad = k_tiles[t] * P - n
            if pad and n:
                idx_f[o + n:o + n + pad] = s[b - 1]   # repeat last src: locality
        esn = np.zeros((t_tiles, P), dtype=np.float32)
        for t in range(t_tiles):
            lo = c * sh + t * P
            hi = min(lo + P, (c + 1) * sh)
            esn[t, :hi - lo] = snrm_all[lo:hi]
        per_core.append({
            "eidx": np.ascontiguousarray(idx_f.reshape(k_total, P).T),
            "edst": np.ascontiguousarray(dst_f.reshape(k_total, P).T),
            "enrm": np.ascontiguousarray(nrm_f.reshape(k_total, P).T),
            "esnrm": np.ascontiguousarray(esn.T),
        })
    return per_core, k_tiles


# ---------------------------------------------------------------- program
def build_program(n_nodes, n_layers, n_cls, k_tiles, ncores=NCORES):
    f32 = mybir.dt.float32
    bf16 = mybir.dt.bfloat16
    sh = n_nodes // ncores
    t_tiles = math.ceil(sh / P)
    k_total = sum(k_tiles)
    offs = np.concatenate([[0], np.cumsum(k_tiles)])
    groups = [list(range(g * G_TILES, min((g + 1) * G_TILES, t_tiles)))
              for g in range(math.ceil(t_tiles / G_TILES))]

    nc = bacc.Bacc("TRN2", target_bir_lowering=False, debug=False,
                   num_devices=ncores)
    xb_t = nc.dram_tensor("xbf", [n_nodes, P], bf16, kind="ExternalInput")
    xs_t = nc.dram_tensor("xself", [sh, P], bf16, kind="ExternalInput")
    idx_t = nc.dram_tensor("eidx", [P, k_total], mybir.dt.int32, kind="ExternalInput")
    dst_t = nc.dram_tensor("edst", [P, k_total], f32, kind="ExternalInput")
    nrm_t = nc.dram_tensor("enrm", [P, k_total], f32, kind="ExternalInput")
    sn_t = nc.dram_tensor("esnrm", [P, t_tiles], f32, kind="ExternalInput")
    w_t = nc.dram_tensor("conv_w", [n_layers, P, P], f32, kind="ExternalInput")
    cb_t = nc.dram_tensor("conv_b", [n_layers, P], f32, kind="ExternalInput")
    gam_t = nc.dram_tensor("bn_gamma", [n_layers, P], f32, kind="ExternalInput")
    bet_t = nc.dram_tensor("bn_beta", [n_layers, P], f32, kind="ExternalInput")
    mu_t = nc.dram_tensor("bn_mean", [n_layers, P], f32, kind="ExternalInput")
    var_t = nc.dram_tensor("bn_var", [n_layers, P], f32, kind="ExternalInput")
    lw_t = nc.dram_tensor("lin_w", [P, n_cls], f32, kind="ExternalInput")
    lb_t = nc.dram_tensor("lin_b_rep", [P, n_cls], f32, kind="ExternalInput")
    out_t = nc.dram_tensor("out", [sh, n_cls], f32, kind="ExternalOutput")

    hbuf = [nc.dram_tensor(f"hbuf{l}", [n_nodes, P], bf16, addr_space="Shared")
            for l in range(1, n_layers)]
    hbuf = [xb_t] + hbuf
    ag_in = [nc.dram_tensor(f"ag_in{l}", [sh, P], bf16)
             for l in range(n_layers - 1)]
    rgroups = [list(range(ncores))]
    AF = mybir.ActivationFunctionType
    OP = mybir.AluOpType

    with tile.TileContext(nc) as tc:
        with tc.tile_pool(name="const", bufs=1) as cpool, \
             tc.tile_pool(name="edges", bufs=1) as epool, \
             tc.tile_pool(name="msgs", bufs=10) as mpool, \
             tc.tile_pool(name="spool", bufs=8) as spool, \
             tc.tile_pool(name="work", bufs=3) as wpool, \
             tc.tile_pool(name="psum", bufs=2, space="PSUM") as pspool, \
             tc.tile_pool(name="psumq", bufs=4, space="PSUM") as pqpool:

            # -------- resident edge data + constants
            idx_sb = epool.tile([P, k_total], mybir.dt.int32)
            dst_sb = epool.tile([P, k_total], f32)
            nrm_sb = epool.tile([P, k_total], f32)
            sn_sb = epool.tile([P, t_tiles], f32)
            nc.sync.dma_start(out=idx_sb[:], in_=idx_t[:])
            nc.sync.dma_start(out=dst_sb[:], in_=dst_t[:])
            nc.sync.dma_start(out=nrm_sb[:], in_=nrm_t[:])
            nc.sync.dma_start(out=sn_sb[:], in_=sn_t[:])

            iota_i = cpool.tile([P, P], mybir.dt.int32)
            nc.gpsimd.iota(iota_i[:], pattern=[[1, P]], base=0, channel_multiplier=0)
            iota_b = cpool.tile([P, P], bf16)
            nc.vector.tensor_copy(iota_b[:], iota_i[:])
            lane_i = cpool.tile([P, 1], mybir.dt.int32)
            nc.gpsimd.iota(lane_i[:], pattern=[[0, 1]], base=0, channel_multiplier=1)
            lane_f = cpool.tile([P, 1], f32)
            nc.vector.tensor_copy(lane_f[:], lane_i[:])
            ident = cpool.tile([P, P], bf16)
            make_identity(nc, ident[:])

            w_sb = []
            for l in range(n_layers):
                wf = cpool.tile([P, P], f32, tag=f"wf{l}")
                nc.sync.dma_start(out=wf[:], in_=w_t[l, :, :])
                wl = cpool.tile([P, P], bf16, tag=f"w{l}")
                nc.vector.tensor_copy(wl[:], wf[:])
                w_sb.append(wl)
            lwf = cpool.tile([P, n_cls], f32)
            nc.sync.dma_start(out=lwf[:], in_=lw_t[:])
            lw_sb = cpool.tile([P, n_cls], bf16)
            nc.vector.tensor_copy(lw_sb[:], lwf[:])
            lb_sb = cpool.tile([P, n_cls], f32)
            nc.sync.dma_start(out=lb_sb[:], in_=lb_t[:])

            # -------- BN constants per layer: scale s = gamma * rsqrt(var+eps)
            #          shift = s*(conv_b - mean) + beta      (feature-major [P,1])
            s_sb, sh_sb = [], []
            for l in range(n_layers):
                g_ = cpool.tile([P, 1], f32, tag=f"bng{l}")
                b_ = cpool.tile([P, 1], f32, tag=f"bnb{l}")
                m_ = cpool.tile([P, 1], f32, tag=f"bnm{l}")
                v_ = cpool.tile([P, 1], f32, tag=f"bnv{l}")
                cb_ = cpool.tile([P, 1], f32, tag=f"bnc{l}")
                nc.sync.dma_start(out=g_[:], in_=gam_t[l, :, None])
                nc.sync.dma_start(out=b_[:], in_=bet_t[l, :, None])
                nc.sync.dma_start(out=m_[:], in_=mu_t[l, :, None])
                nc.sync.dma_start(out=v_[:], in_=var_t[l, :, None])
                nc.sync.dma_start(out=cb_[:], in_=cb_t[l, :, None])
                ve = cpool.tile([P, 1], f32, tag=f"bnve{l}")
                nc.vector.tensor_scalar_add(ve[:], v_[:], BN_EPS)
                nc.scalar.sqrt(ve[:], ve[:])
                rv = cpool.tile([P, 1], f32, tag=f"bnrv{l}")
                nc.vector.reciprocal(rv[:], ve[:])
                s_ = cpool.tile([P, 1], f32, tag=f"bns{l}")
                nc.vector.tensor_tensor(out=s_[:], in0=g_[:], in1=rv[:], op=OP.mult)
                d_ = cpool.tile([P, 1], f32, tag=f"bnd{l}")
                nc.vector.tensor_tensor(out=d_[:], in0=cb_[:], in1=m_[:], op=OP.subtract)
                t_ = cpool.tile([P, 1], f32, tag=f"bnt{l}")
                nc.vector.tensor_tensor(out=t_[:], in0=d_[:], in1=s_[:], op=OP.mult)
                nc.vector.tensor_tensor(out=t_[:], in0=t_[:], in1=b_[:], op=OP.add)
                s_sb.append(s_)
                sh_sb.append(t_)

            hmax = epool.tile([P, t_tiles * P], bf16)
            nc.vector.memset(hmax[:], 0.0)

            # -------- layers
            for l in range(n_layers):
                table = hbuf[l]
                table_self = xs_t if l == 0 else ag_in[l - 1]
                for grp in groups:
                    for t in grp:
                        kt = k_tiles[t]
                        rows = min(P, sh - t * P)
                        psq = pqpool.tile([P, P], f32, tag="q", space="PSUM")
                        # self-loop chunk: contiguous bulk read of own shard
                        # rows (HWDGE - no per-row descriptor cost)
                        ms = mpool.tile([P, P], bf16, tag="ms")
                        nc.sync.dma_start(out=ms[:rows, :],
                                          in_=table_self[t * P:t * P + rows, :])
                        ss = spool.tile([P, P], bf16, tag="ss")
                        nc.vector.tensor_scalar(
                            out=ss[:], in0=iota_b[:],
                            scalar1=lane_f[:, :1],
                            scalar2=sn_sb[:, t:t + 1],
                            op0=OP.is_equal, op1=OP.mult)
                        nc.tensor.matmul(psq[:], lhsT=ms[:], rhs=ss[:],
                                         start=True, stop=False)
                        for j in range(kt):
                            c = int(offs[t]) + j
                            msgs = mpool.tile([P, P], bf16, tag="msgs")
                            nc.gpsimd.indirect_dma_start(
                                out=msgs[:], out_offset=None,
                                in_=table[:],
                                in_offset=bass.IndirectOffsetOnAxis(
                                    ap=idx_sb[:, c:c + 1], axis=0),
                            )
                            s_tile = spool.tile([P, P], bf16, tag="S")
                            nc.vector.tensor_scalar(
                                out=s_tile[:], in0=iota_b[:],
                                scalar1=dst_sb[:, c:c + 1],
                                scalar2=nrm_sb[:, c:c + 1],
                                op0=OP.is_equal, op1=OP.mult)
                            nc.tensor.matmul(
                                psq[:], lhsT=msgs[:],
                                rhs=s_tile[:], start=False, stop=(j == kt - 1))
                        q_sb = wpool.tile([P, P], bf16, tag="qT")
                        nc.scalar.copy(q_sb[:], psq[:])
                        ph = pspool.tile([P, P], f32, tag="h", space="PSUM")
                        nc.tensor.matmul(ph[:], lhsT=w_sb[l][:], rhs=q_sb[:],
                                         start=True, stop=True)
                        h_t = wpool.tile([P, P], bf16, tag="hT")
                        nc.scalar.activation(h_t[:], ph[:], AF.Relu,
                                             bias=sh_sb[l][:, :1],
                                             scale=s_sb[l][:, :1])
                        nc.vector.tensor_tensor(
                            out=hmax[:, t * P:(t + 1) * P],
                            in0=hmax[:, t * P:(t + 1) * P], in1=h_t[:], op=OP.max)
                        if l < n_layers - 1:
                            pt = pspool.tile([P, P], bf16, tag="t", space="PSUM")
                            nc.tensor.transpose(pt[:], h_t[:], ident[:])
                            hn = wpool.tile([P, P], bf16, tag="hn")
                            nc.scalar.copy(hn[:], pt[:])
                            nc.sync.dma_start(out=ag_in[l][t * P:t * P + rows, :],
                                              in_=hn[:rows, :])
                if l < n_layers - 1:
                    nc.gpsimd.collective_compute(
                        "AllGather", OP.bypass, replica_groups=rgroups,
                        ins=[ag_in[l][:]], outs=[hbuf[l + 1][:]])

            # -------- head: logits + log_softmax
            for t in range(t_tiles):
                po = pspool.tile([P, n_cls], f32, tag="h", space="PSUM")
                nc.tensor.matmul(po[:], lhsT=hmax[:, t * P:(t + 1) * P],
                                 rhs=lw_sb[:], start=True, stop=True)
                z = wpool.tile([P, n_cls], f32, tag="z")
                nc.vector.tensor_tensor(out=z[:], in0=po[:], in1=lb_sb[:], op=OP.add)
                nm = wpool.tile([P, 1], f32, tag="nm")
                nc.vector.reduce_max(nm[:], z[:], axis=mybir.AxisListType.X,
                                     negate=True)
                ez = wpool.tile([P, n_cls], f32, tag="ez")
                nc.scalar.activation(ez[:], z[:], AF.Exp, bias=nm[:, :1], scale=1.0)
                ss2 = wpool.tile([P, 1], f32, tag="ss2")
                nc.vector.reduce_sum(ss2[:], ez[:], axis=mybir.AxisListType.X)
                ls = wpool.tile([P, 1], f32, tag="ls")
                nc.scalar.activation(ls[:], ss2[:], AF.Ln)
                oz = wpool.tile([P, n_cls], f32, tag="oz")
                nc.vector.tensor_scalar(out=oz[:], in0=z[:],
                                        scalar1=nm[:, :1], scalar2=ls[:, :1],
                                        op0=OP.add, op1=OP.subtract)
                rows = min(P, sh - t * P)
                nc.sync.dma_start(out=out_t[t * P:t * P + rows, :],
                                  in_=oz[:rows, :])

    nc.compile()
    return nc


# ---------------------------------------------------------------- runner
def run(x, edge_index, conv_w, conv_b, bn_gamma, bn_beta, bn_mean, bn_var,
        lin_w, lin_b, *, trace=False):
    n_nodes, d = x.shape
    n_layers = conv_w.shape[0]
    n_cls = lin_w.shape[1]
    assert d == P and n_nodes % NCORES == 0
    sh = n_nodes // NCORES

    per_core, k_tiles = preprocess_edges(edge_index, n_nodes)
    nc = build_program(n_nodes, n_layers, n_cls, k_tiles)

    from ml_dtypes import bfloat16
    xbf = np.ascontiguousarray(
        np.asarray(x, dtype=np.float32).astype(bfloat16))
    shared = {
        "xbf": xbf,
        "conv_w": np.ascontiguousarray(np.asarray(conv_w, dtype=np.float32)),
        "conv_b": np.ascontiguousarray(np.asarray(conv_b, dtype=np.float32)),
        "bn_gamma": np.ascontiguousarray(np.asarray(bn_gamma, dtype=np.float32)),
        "bn_beta": np.ascontiguousarray(np.asarray(bn_beta, dtype=np.float32)),
        "bn_mean": np.ascontiguousarray(np.asarray(bn_mean, dtype=np.float32)),
        "bn_var": np.ascontiguousarray(np.asarray(bn_var, dtype=np.float32)),
        "lin_w": np.ascontiguousarray(np.asarray(lin_w, dtype=np.float32)),
        "lin_b_rep": np.ascontiguousarray(
            np.broadcast_to(np.asarray(lin_b, dtype=np.float32), (P, n_cls))),
    }
    in_maps = [dict(shared, xself=np.ascontiguousarray(xbf[c * sh:(c + 1) * sh]),
                    **per_core[c])
               for c in range(NCORES)]
    res = run_bass_kernel_spmd(nc, in_maps, list(range(NCORES)), trace=trace)
    out = np.concatenate([np.asarray(res.results[c]["out"])
                          for c in range(NCORES)], axis=0)
    return out, res


def kernel(x, edge_index, conv_w, conv_b, bn_gamma, bn_beta, bn_mean, bn_var,
           lin_w, lin_b):
    out, _ = run(x, edge_index, conv_w, conv_b, bn_gamma, bn_beta,
                 bn_mean, bn_var, lin_w, lin_b,
                 trace=bool(int(os.environ.get("JKNET_TRACE", "0"))))
    return out
